# revision 45
# baseline (speedup 1.0000x reference)
"""Trainium2 Bass kernel for the FEM dual-attention module (v3).

Full (unsharded) inputs in, full outputs (E_q, E_s) out. Data-parallel over
batch B=16 across 8 NeuronCores (2 samples each); BatchNorm batch statistics
combined with one in-kernel AllGather + local sum.

Structure (vs the reference dataflow):
  * att folded into output weights: t = p^T W = v^T (att^T W) = v^T Weff —
    p_s / q_s never materialized.
  * BN statistics from Gram matrices: sum_c = sv^T Weff,
    ssq_c = Weff[:,c]^T (v v^T) Weff[:,c]; the Gram runs as fp8e4m3
    DoubleRow matmuls (2x PE rate).
  * fp16 operands elsewhere (fp32 PSUM accumulation); fp16 host I/O staging.
  * k^T/q^T produced directly in token-major layout (x chunk as stationary).
  * Channel-gate pooling: DMA-accumulate folds x 4096->1024 on the Pool
    queue's CCE, then a short DVE add-tree (reduce gets no 2x mode; adds do).
  * Finals: PSUM = (sc2-scaled Weff)^T v + I @ x, so the single drain is
    copy+bias(sh2) — BN scale, shift, and residual all land in one pass.
"""

import numpy as np

import concourse.bass as bass
import concourse.mybir as mybir
import concourse.tile as tile
from concourse import bacc
from concourse.bass_utils import run_bass_kernel_spmd
from concourse.masks import make_identity

B, C, N, IC, R = 16, 320, 4096, 128, 4
EPS = 1e-5
NCORES = 8
BPC = B // NCORES
P = 128
NT = 8
TS = 512
G = C // R
CCH = [(0, 128), (128, 128), (256, 64)]
ROWS_TOTAL = float(B * N)

F32 = mybir.dt.float32
F16 = mybir.dt.float16
F8 = mybir.dt.float8e4
AX = mybir.AxisListType.X
AF = mybir.ActivationFunctionType
ALU = mybir.AluOpType
DR = mybir.MatmulPerfMode.DoubleRow

_CACHE = {}


def _pcx(o):
    return 65 if o == 2 else CCH[o][1]


def build_program(reps=1):
    nc = bacc.Bacc("TRN2", target_bir_lowering=False, debug=False,
                   num_devices=NCORES)

    q_loc = nc.dram_tensor("q_loc", [BPC, C, N], F16, kind="ExternalInput").ap()
    s_loc = nc.dram_tensor("s_loc", [BPC, C, N], F16, kind="ExternalInput").ap()
    Wv = nc.dram_tensor("Wv", [C, IC], F16, kind="ExternalInput").ap()
    bv = nc.dram_tensor("bv", [IC], F16, kind="ExternalInput").ap()
    Wk = nc.dram_tensor("Wk", [C, IC], F16, kind="ExternalInput").ap()
    bk = nc.dram_tensor("bk", [IC], F16, kind="ExternalInput").ap()
    Wqp = nc.dram_tensor("Wqp", [C, IC], F16, kind="ExternalInput").ap()
    bqp = nc.dram_tensor("bqp", [IC], F16, kind="ExternalInput").ap()
    Wts = nc.dram_tensor("Wts", [IC, C], F16, kind="ExternalInput").ap()
    Wtq = nc.dram_tensor("Wtq", [IC, C], F16, kind="ExternalInput").ap()
    gts = nc.dram_tensor("gts", [C], F32, kind="ExternalInput").ap()
    bets = nc.dram_tensor("bets", [C], F32, kind="ExternalInput").ap()
    gtq = nc.dram_tensor("gtq", [C], F32, kind="ExternalInput").ap()
    betq = nc.dram_tensor("betq", [C], F32, kind="ExternalInput").ap()
    Wg1 = nc.dram_tensor("Wg1", [C, G], F16, kind="ExternalInput").ap()
    bg1 = nc.dram_tensor("bg1", [G], F16, kind="ExternalInput").ap()
    Wg2 = nc.dram_tensor("Wg2", [G, C], F16, kind="ExternalInput").ap()
    bg2 = nc.dram_tensor("bg2", [C], F32, kind="ExternalInput").ap()
    eq_loc = nc.dram_tensor("eq_loc", [BPC, C, N], F16, kind="ExternalOutput").ap()
    es_loc = nc.dram_tensor("es_loc", [BPC, C, N], F16, kind="ExternalOutput").ap()

    with tile.TileContext(nc) as tc:
        nc._lp_ctx = nc.allow_low_precision(
            reason="fp16/fp8 matmul operands and drains; fp32 accumulation")
        nc._lp_ctx.__enter__()
        with (
            tc.tile_pool(name="singles", bufs=1) as singles,
            tc.tile_pool(name="resident", bufs=1) as resident,
            tc.tile_pool(name="streams", bufs=2) as streams,
            tc.tile_pool(name="smalls", bufs=2) as smalls,
            tc.tile_pool(name="dram", bufs=1, space="DRAM") as dram,
        ):
            for _ in range(reps):
                emit_body(nc, tc, singles, resident, streams, smalls, dram,
                          q_loc, s_loc, Wv, bv, Wk, bk, Wqp, bqp, Wts, Wtq,
                          gts, bets, gtq, betq, Wg1, bg1, Wg2, bg2,
                          eq_loc, es_loc)

    nc.compile()
    return nc


def emit_body(nc, tc, singles, resident, streams, smalls, dram,
              q_loc, s_loc, Wv, bv, Wk, bk, Wqp, bqp, Wts, Wtq,
              gts, bets, gtq, betq, Wg1, bg1, Wg2, bg2, eq_loc, es_loc):

    def load_kxm_aug(w_ap, b_ap, m, tag):
        t = singles.tile([P, 3, m], F16, tag=tag, name=f"w_{tag}")
        nc.sync.dma_start(
            t[:, 0:2, :], w_ap[0:256, :].rearrange("(o p) i -> p o i", p=P))
        nc.sync.dma_start(t[:64, 2, :], w_ap[256:C, :])
        nc.sync.dma_start(t[64:65, 2, :], b_ap.unsqueeze(0))
        return t

    def load_cvec(v_ap, tag):
        t = singles.tile([P, 3], F32, tag=tag, name=f"v_{tag}")
        nc.gpsimd.memset(t[:], 0.0)
        nc.sync.dma_start(t[:, 0:2], v_ap[0:256].rearrange("(o p) -> p o", p=P))
        nc.sync.dma_start(t[:64, 2:3], v_ap[256:C].unsqueeze(1))
        return t

    Wv_t = load_kxm_aug(Wv, bv, IC, "wv")
    Wk_t = load_kxm_aug(Wk, bk, IC, "wk")
    Wq_t = load_kxm_aug(Wqp, bqp, IC, "wq")

    ident16 = singles.tile([P, P], F16, tag="ident16")
    make_identity(nc, ident16[:])
    ident32 = singles.tile([P, P], F32, tag="ident32")
    make_identity(nc, ident32[:])
    eps_t = singles.tile([P, 1], F32, tag="eps")
    nc.vector.memset(eps_t[:], EPS)

    # ---------------- resident x tiles + loads ----------------
    xs_t, xq_t = [], []
    for b in range(BPC):
        xs = resident.tile([P, 3, N], F16, tag=f"xs{b}", name=f"xs{b}")
        xq = resident.tile([P, 3, N], F16, tag=f"xq{b}", name=f"xq{b}")
        xs_t.append(xs)
        xq_t.append(xq)
    nc.gpsimd.memset(xs_t[0][64:128, 2, :], 1.0)
    nc.vector.memset(xq_t[0][64:128, 2, :], 1.0)
    nc.gpsimd.memset(xs_t[1][64:128, 2, :], 1.0)
    nc.vector.memset(xq_t[1][64:128, 2, :], 1.0)
    # xs on the SP queue, xq on the Pool queue (quarters so first tiles
    # land early); the two queues stream the input tensors in parallel
    for b in range(BPC):
        nq = 4 if b == 0 else 2
        for h in range(nq):
            ns = slice(h * (N // nq), (h + 1) * (N // nq))
            for tn, (src, dst) in enumerate(((s_loc, xs_t[b]),
                                             (q_loc, xq_t[b]))):
                eng = nc.gpsimd if tn == 1 else nc.sync
                eng.dma_start(
                    dst[:, 0:2, ns],
                    src[b, 0:256, ns].rearrange("(o p) n -> p o n", p=P))
                eng.dma_start(dst[:64, 2, ns], src[b, 256:C, ns])

    Wts_t = singles.tile([P, C], F16, tag="wts")
    nc.sync.dma_start(Wts_t[:], Wts[:, :])
    Wtq_t = singles.tile([P, C], F16, tag="wtq")
    nc.sync.dma_start(Wtq_t[:], Wtq[:, :])
    Wg1_t = load_kxm_aug(Wg1, bg1, G, "wg1")
    Wg2_t = singles.tile([G, C], F16, tag="wg2")
    nc.sync.dma_start(Wg2_t[:], Wg2[:, :])
    bg2_t = load_cvec(bg2, "bg2")
    gts_t = load_cvec(gts, "gts")
    bets_t = load_cvec(bets, "bets")
    gtq_t = load_cvec(gtq, "gtq")
    betq_t = load_cvec(betq, "betq")

    # pooled fold for the s tensors: x [128,3,4096] -> [128,3,1024] via the
    # Pool queue CCE; q tensors are tree-summed on DVE in the gather window
    xfold = {}
    for b in range(BPC):
        xt = xs_t[b]
        xf = smalls.tile([P, 3, 1024], F16, tag="xfold", bufs=4,
                         name=f"xfold{b}0")
        nc.gpsimd.dma_start(xf[:], xt[:, :, 0:1024])
        for u in range(1, 4):
            nc.gpsimd.dma_start(xf[:], xt[:, :, u * 1024:(u + 1) * 1024],
                                accum_op=ALU.add)
        xfold[(b, 0)] = xf

    acc = smalls.tile([P, 24], F32, tag="acc", bufs=1)
    nc.vector.memset(acc[:], 0.0)
    pooled4 = smalls.tile([P, 3, 4], F16, tag="pooled4", bufs=1)
    nc.vector.memset(pooled4[:], 0.0)
    nc.vector.memset(pooled4[64:65, 2, :], 1.0)

    v_sb, Weff_sb, WeffT_sb, Gv_sb, MT_sb = {}, {}, {}, {}, {}

    with tc.tile_pool(name="ps1", bufs=1, space="PSUM") as ps1:

        def ps_tile(tag, shape, dtype, bufs):
            return ps1.tile(shape, dtype, tag=tag, bufs=bufs, name=f"ps_{tag}")

        for b in range(BPC):
            xs, xq = xs_t[b], xq_t[b]
            pers = ps_tile("pers", [P, 512], F32, 1)
            psA = pers[:, 0:128]
            psGs = pers[:, 128:256]
            psGq = pers[:, 256:384]
            psAT = pers[:, 384:512]

            for t in range(2):
                v_sb[(b, t)] = resident.tile([P, NT, TS], F16,
                                             tag=f"v{b}{t}", name=f"v{b}{t}")
            svcols_s = smalls.tile([P, NT], F32, tag="svs", name="svs")
            svcols_q = smalls.tile([P, NT], F32, tag="svq", name="svq")

            def gram(nt_, kT_, qT_, vT_):
                for u in range(2):
                    nc.tensor.matmul(psGs, vT_[:, 2 * u:2 * u + 2, :],
                                     vT_[:, 2 * u:2 * u + 2, :],
                                     start=(nt_ == 0 and u == 0),
                                     stop=(nt_ == NT - 1 and u == 1),
                                     perf_mode=DR, skip_group_check=True)
                for u in range(2):
                    nc.tensor.matmul(psGq, vT_[:, 4 + 2 * u:6 + 2 * u, :],
                                     vT_[:, 4 + 2 * u:6 + 2 * u, :],
                                     start=(nt_ == 0 and u == 0),
                                     stop=(nt_ == NT - 1 and u == 1),
                                     perf_mode=DR, skip_group_check=True)
                for u in range(4):
                    nc.tensor.matmul(psA, kT_[:, u, :], qT_[:, u, :],
                                     start=(nt_ == 0 and u == 0),
                                     stop=(nt_ == NT - 1 and u == 3),
                                     skip_group_check=True)

            prev = None
            for nt in range(NT):
                ns = slice(nt * TS, (nt + 1) * TS)
                ps_vs = ps_tile("vs", [P, TS], F32, 2)
                ps_vq = ps_tile("vq", [P, TS], F32, 2)
                for o in range(3):
                    nc.tensor.matmul(ps_vs[:], Wv_t[:_pcx(o), o, :],
                                     xs[:_pcx(o), o, ns],
                                     start=(o == 0), stop=(o == 2))
                for o in range(3):
                    nc.tensor.matmul(ps_vq[:], Wv_t[:_pcx(o), o, :],
                                     xq[:_pcx(o), o, ns],
                                     start=(o == 0), stop=(o == 2))
                ps_kT = ps_tile("kT", [P, 4, P], F32, 1)
                ps_qT = ps_tile("qT", [P, 4, P], F32, 1)
                for u in range(4):
                    tc_sl = slice(nt * TS + u * P, nt * TS + (u + 1) * P)
                    for o in range(3):
                        nc.tensor.matmul(ps_kT[:, u, :], xs[:_pcx(o), o, tc_sl],
                                         Wk_t[:_pcx(o), o, :],
                                         start=(o == 0), stop=(o == 2))
                for u in range(4):
                    tc_sl = slice(nt * TS + u * P, nt * TS + (u + 1) * P)
                    for o in range(3):
                        nc.tensor.matmul(ps_qT[:, u, :], xq[:_pcx(o), o, tc_sl],
                                         Wq_t[:_pcx(o), o, :],
                                         start=(o == 0), stop=(o == 2))

                # drains: v on ACT (with sv accum), kqT + vT(fp8) on DVE
                nc.scalar.activation(v_sb[(b, 0)][:, nt, :], ps_vs[:],
                                     AF.Identity,
                                     accum_out=svcols_s[:, nt:nt + 1])
                nc.scalar.activation(v_sb[(b, 1)][:, nt, :], ps_vq[:],
                                     AF.Identity,
                                     accum_out=svcols_q[:, nt:nt + 1])
                kT_sb = streams.tile([P, 4, P], F16, tag="kT_sb")
                qT_sb = streams.tile([P, 4, P], F16, tag="qT_sb")
                nc.vector.tensor_copy(kT_sb[:], ps_kT[:])
                nc.scalar.copy(qT_sb[:], ps_qT[:])

                if prev is not None:
                    gram(*prev)

                ps_vT = ps_tile("vT", [P, 8, P], F16, 1)
                for u in range(4):
                    nc.tensor.transpose(
                        ps_vT[:, u, :],
                        v_sb[(b, 0)][:, nt, u * P:(u + 1) * P], ident16[:])
                for u in range(4):
                    nc.tensor.transpose(
                        ps_vT[:, 4 + u, :],
                        v_sb[(b, 1)][:, nt, u * P:(u + 1) * P], ident16[:])
                vT_sb = streams.tile([P, 8, P], F8, tag="vT_sb")
                nc.vector.tensor_copy(vT_sb[:], ps_vT[:])

                prev = (nt, kT_sb, qT_sb, vT_sb)
            gram(*prev)

            # -------- sample tail: softmax + Weff + Gv drains --------
            A_sb = smalls.tile([P, P], F32, tag="A_sb", name="A_sb")
            nc.scalar.copy(A_sb[:], psA)

            def softmax(ps_src, tag):
                negm = smalls.tile([P, 1], F32, tag=f"negm{tag}", name=f"nm{tag}")
                nc.vector.reduce_max(negm[:], ps_src, axis=AX, negate=True)
                e = smalls.tile([P, P], F16, tag=f"e{tag}", name=f"e{tag}")
                nc.scalar.activation(e[:], ps_src, AF.Exp, bias=negm[:], scale=1.0)
                ssum = smalls.tile([P, 1], F32, tag=f"ss{tag}", name=f"ss{tag}")
                nc.vector.reduce_sum(ssum[:], e[:], axis=AX)
                rinv = smalls.tile([P, 1], F32, tag=f"ri{tag}", name=f"ri{tag}")
                nc.vector.reciprocal(rinv[:], ssum[:])
                att = smalls.tile([P, P], F16, tag=f"att{tag}", name=f"att{tag}")
                nc.vector.tensor_scalar_mul(att[:], e[:], rinv[:])
                return att

            att_s = softmax(psA, "s")
            nc.tensor.transpose(psAT, A_sb[:], ident32[:])
            att_q = softmax(psAT, "q")

            for path, (att, w_t) in enumerate(((att_s, Wts_t), (att_q, Wtq_t))):
                psW = ps_tile(["kT", "qT"][path], [P, C], F32, 1)
                nc.tensor.matmul(psW[:], att[:], w_t[:])
                Weff_sb[(b, path)] = smalls.tile([P, C], F16, tag=f"weff{path}",
                                                 name=f"weff{b}{path}")
                nc.scalar.copy(Weff_sb[(b, path)][:], psW[:])

            for t, (psG, svc) in enumerate(((psGs, svcols_s), (psGq, svcols_q))):
                gv = smalls.tile([P, IC + 1], F16, tag=f"gv{t}", name=f"gv{b}{t}")
                nc.scalar.copy(gv[:, :IC], psG)
                sv = smalls.tile([P, 1], F32, tag=f"sv{t}", name=f"sv{b}{t}")
                nc.vector.reduce_sum(sv[:], svc[:], axis=AX)
                nc.vector.tensor_copy(gv[:, IC:IC + 1], sv[:])
                Gv_sb[(b, t)] = gv

        pans = [(b, path) for b in range(BPC) for path in range(2)]
        # stats tail for both samples
        junk = smalls.tile([P, P], F32, tag="junk", bufs=1)
        for g, (b, path) in enumerate(pans):
            psWT = ps_tile(["vs", "vs", "vq", "vq"][g], [P, 3, P], F16, 2)
            for o, (c0, pc) in enumerate(CCH):
                nc.tensor.transpose(psWT[:pc, o, :],
                                    Weff_sb[(b, path)][:, c0:c0 + pc],
                                    ident16[:])
            WeffT_sb[(b, path)] = smalls.tile([P, 3, P], F16, tag="wefft",
                                              bufs=4, name=f"wefft{b}{path}")
            for o, (c0, pc) in enumerate(CCH):
                nc.vector.tensor_copy(WeffT_sb[(b, path)][:pc, o, :],
                                      psWT[:pc, o, :])
            psMT = ps_tile(["kT", "qT", "kT", "qT"][g], [P, 3, IC + 1], F32, 1)
            for o, (c0, pc) in enumerate(CCH):
                nc.tensor.matmul(psMT[:pc, o, :],
                                 Weff_sb[(b, path)][:, c0:c0 + pc],
                                 Gv_sb[(b, path)][:])
            MT_sb[(b, path)] = smalls.tile([P, 3, IC + 1], F16, tag="mt",
                                           bufs=4, name=f"mt{b}{path}")
            for o, (c0, pc) in enumerate(CCH):
                nc.scalar.copy(MT_sb[(b, path)][:pc, o, :], psMT[:pc, o, :])
            mt, wt = MT_sb[(b, path)], WeffT_sb[(b, path)]
            for o, (c0, pc) in enumerate(CCH):
                sumcol = 12 * b + path * 6 + o
                ssqcol = 12 * b + path * 6 + 3 + o
                nc.vector.tensor_mul(junk[:pc, :], mt[:pc, o, :IC],
                                     wt[:pc, o, :])
                nc.vector.reduce_sum(acc[:pc, ssqcol:ssqcol + 1],
                                     junk[:pc, :], axis=AX)
                nc.vector.tensor_copy(acc[:pc, sumcol:sumcol + 1],
                                      mt[:pc, o, IC:IC + 1])
        # local sample-sum -> [128, 12]
        acc12 = smalls.tile([P, 12], F32, tag="acc12", bufs=1)
        nc.vector.tensor_add(acc12[:], acc[:, 0:12], acc[:, 12:24])

        # AllGather the per-core stats, sum the 8 slots locally
        # AllGather writes slot-contiguous [slot][partition][col] blocks
        cc_in = dram.tile([P, 12], F32, name="cc_in")
        cc_out = dram.tile([NCORES, P, 12], F32, name="cc_out")
        nc.gpsimd.dma_start(cc_in[:], acc12[:])
        nc.gpsimd.collective_compute(
            "AllGather", ALU.bypass,
            replica_groups=[list(range(NCORES))],
            ins=[cc_in.opt()], outs=[cc_out.opt()],
        )
        ag = smalls.tile([P, 8, 12], F32, tag="ag", bufs=1)
        for gslot in range(NCORES):
            nc.gpsimd.dma_start(ag[:, gslot, :], cc_out[gslot])

        # ---- AllGather window: pooled add-trees + gate MLP ----
        for b in range(BPC):
            for t in range(2):
                if t == 1:
                    # q: pure DVE tree from the resident tensor
                    xq = xq_t[b]
                    xf = smalls.tile([P, 3, 1024], F16, tag="xfold", bufs=4,
                                     name=f"qs1_{b}")
                    s2 = smalls.tile([P, 3, 1024], F16, tag="xfold", bufs=4,
                                     name=f"qs2_{b}")
                    nc.vector.tensor_add(xf[:], xq[:, :, 0:1024],
                                         xq[:, :, 1024:2048])
                    nc.vector.tensor_add(s2[:], xq[:, :, 2048:3072],
                                         xq[:, :, 3072:4096])
                    nc.vector.tensor_add(xf[:], xf[:], s2[:])
                else:
                    xf = xfold[(b, t)]
                w = 512
                while w >= 8:
                    nc.vector.tensor_add(xf[:, :, 0:w], xf[:, :, 0:w],
                                         xf[:, :, w:2 * w])
                    w //= 2
                sx = smalls.tile([P, 3], F16, tag="sxs", bufs=4,
                                 name=f"sxs{b}{t}")
                nc.vector.reduce_sum(sx[:], xf[:, :, 0:8], axis=AX)
                col = b * 2 + t
                nc.vector.tensor_scalar_mul(pooled4[:, :, col:col + 1],
                                            sx[:].unsqueeze(2), 1.0 / float(N))

        ps_h = ps_tile("vT", [P, 4], F32, 1)
        for o in range(3):
            nc.tensor.matmul(ps_h[:G, :], Wg1_t[:_pcx(o), o, :],
                             pooled4[:_pcx(o), o, :],
                             start=(o == 0), stop=(o == 2))
        h_sb = smalls.tile([G, 4], F16, tag="h_sb", bufs=1)
        nc.scalar.activation(h_sb[:], ps_h[:G, :], AF.Relu)
        gates = smalls.tile([P, 3, 4], F32, tag="gates", bufs=1)
        nc.vector.memset(gates[64:, 2, :], 0.0)
        for o, (c0, pc) in enumerate(CCH):
            ps_g = ps_tile("vT", [P, 4], F32, 1)
            nc.tensor.matmul(ps_g[:pc, :], Wg2_t[:, c0:c0 + pc], h_sb[:])
            nc.scalar.activation(gates[:pc, o, :], ps_g[:pc, :], AF.Sigmoid,
                                 bias=bg2_t[:pc, o:o + 1], scale=1.0)

        # ---- combine gathered stats, BN coefficients ----
        cc_res = smalls.tile([P, 12], F32, tag="cc_res", bufs=1)
        nc.vector.tensor_add(cc_res[:], ag[:, 0, :], ag[:, 1, :])
        for gslot in range(2, 8):
            nc.vector.tensor_add(cc_res[:], cc_res[:], ag[:, gslot, :])

        def bn_coeffs(qoff, g_t, be_t, tag):
            mean = smalls.tile([P, 3], F32, tag=f"mn{tag}", name=f"mn{tag}")
            nc.vector.tensor_scalar_mul(mean[:], cc_res[:, qoff:qoff + 3],
                                        1.0 / ROWS_TOTAL)
            var = smalls.tile([P, 3], F32, tag=f"vr{tag}", name=f"vr{tag}")
            nc.vector.tensor_scalar_mul(var[:], cc_res[:, qoff + 3:qoff + 6],
                                        1.0 / ROWS_TOTAL)
            msq = smalls.tile([P, 3], F32, tag=f"mq{tag}", name=f"mq{tag}")
            nc.vector.tensor_mul(msq[:], mean[:], mean[:])
            nc.vector.tensor_sub(var[:], var[:], msq[:])
            sd = smalls.tile([P, 3], F32, tag=f"sd{tag}", name=f"sd{tag}")
            nc.scalar.activation(sd[:], var[:], AF.Sqrt, bias=eps_t[:], scale=1.0)
            rstd = smalls.tile([P, 3], F32, tag=f"rs{tag}", name=f"rs{tag}")
            nc.vector.reciprocal(rstd[:], sd[:])
            sc = smalls.tile([P, 3], F32, tag=f"sc{tag}", name=f"sc{tag}")
            nc.vector.tensor_mul(sc[:], g_t[:], rstd[:])
            sh = smalls.tile([P, 3], F32, tag=f"sh{tag}", name=f"sh{tag}")
            nc.vector.tensor_mul(sh[:], sc[:], mean[:])
            nc.vector.tensor_sub(sh[:], be_t[:], sh[:])
            return sc, sh

        sc_P, sh_P = bn_coeffs(0, gts_t, bets_t, "P")
        sc_Q, sh_Q = bn_coeffs(6, gtq_t, betq_t, "Q")

    # ---------------- finals (ps1 released; ps2 owns all 8 banks) ----------
    with tc.tile_pool(name="ps2", bufs=1, space="PSUM") as ps2:
        # fold gate*scale into Weff (per path-sample)
        Weff2_sb, sh2_all = {}, {}
        for g, (b, path) in enumerate(pans):
            col = b * 2 + path
            sc, sh = (sc_P, sh_P) if path == 0 else (sc_Q, sh_Q)
            sc2 = smalls.tile([P, 3], F32, tag="sc2", bufs=2, name=f"sc2_{g}")
            sh2 = smalls.tile([P, 3], F32, tag="sh2", bufs=4, name=f"sh2_{g}")
            nc.vector.tensor_mul(sc2[:], sc[:], gates[:, :, col])
            nc.vector.tensor_mul(sh2[:], sh[:], gates[:, :, col])
            sh2_all[g] = sh2
            wt2 = smalls.tile([P, 3, P], F16, tag="wefft2", bufs=2,
                              name=f"wefft2{g}")
            for o in range(3):
                pc = CCH[o][1]
                nc.vector.tensor_scalar_mul(wt2[:pc, o, :],
                                            WeffT_sb[(b, path)][:pc, o, :],
                                            sc2[:pc, o:o + 1])
            psW2 = ps2.tile([P, C], F16, tag="fin", bufs=4, name=f"psW2_{g}")
            for o, (c0, pc) in enumerate(CCH):
                nc.tensor.transpose(psW2[:, c0:c0 + pc], wt2[:pc, o, :],
                                    ident16[:pc, :pc])
            Weff2_sb[g] = smalls.tile([P, C], F16, tag="weff2", bufs=4,
                                      name=f"weff2{g}")
            nc.scalar.copy(Weff2_sb[g][:], psW2[:])

        out_q = [nc.sync, nc.gpsimd]
        for g, (b, path) in enumerate(pans):
            x_res = xs_t[b] if path == 0 else xq_t[b]
            out_ap = es_loc if path == 0 else eq_loc
            weff2 = Weff2_sb[g]
            sh2 = sh2_all[g]
            v_t = v_sb[(b, path)]
            for o, (c0, pc) in enumerate(CCH):
                # residual accumulated on the PE (I @ x); single drain adds sh2
                stage = streams.tile([P, NT, TS], F16, tag="stage", bufs=2,
                                     name=f"stage{g}{o}")
                for pair in range(NT // 2):
                    ps_f = ps2.tile([P, 2, TS], F32, tag="fin", bufs=4,
                                    name=f"fin{g}{o}{pair}")
                    for half in range(2):
                        nt = pair * 2 + half
                        ns = slice(nt * TS, (nt + 1) * TS)
                        nc.tensor.matmul(ps_f[:pc, half, :],
                                         ident16[:pc, :pc], x_res[:pc, o, ns],
                                         start=True, stop=False,
                                         skip_group_check=True)
                    for half in range(2):
                        nt = pair * 2 + half
                        nc.tensor.matmul(ps_f[:pc, half, :],
                                         weff2[:, c0:c0 + pc], v_t[:, nt, :],
                                         start=False, stop=True,
                                         skip_group_check=True)
                    st_sl = stage[:pc, 2 * pair:2 * pair + 2, :]
                    if pair % 2 == 0:
                        nc.scalar.activation(st_sl, ps_f[:pc, :, :],
                                             AF.Identity,
                                             bias=sh2[:pc, o:o + 1], scale=1.0)
                    else:
                        nc.vector.tensor_scalar_add(st_sl, ps_f[:pc, :, :],
                                                    sh2[:pc, o:o + 1])
                out_q[(g * 3 + o) % 2].dma_start(
                    out_ap[b, c0:c0 + pc, :].rearrange("p (t n) -> p t n", n=TS),
                    stage[:pc, :, :])


def _get_nc():
    if "nc" not in _CACHE:
        _CACHE["nc"] = build_program()
    return _CACHE["nc"]


def make_in_maps(inputs):
    f16 = np.float16
    q = np.ascontiguousarray(inputs["q"]).astype(f16)
    s = np.ascontiguousarray(inputs["s"]).astype(f16)
    w16 = {k: np.ascontiguousarray(inputs[k]).astype(f16)
           for k in ["Wv", "bv", "Wk", "bk", "Wqp", "bqp", "Wts", "Wtq",
                     "Wg1", "bg1", "Wg2"]}
    w32 = {k: np.ascontiguousarray(inputs[k], dtype=np.float32)
           for k in ["gts", "bets", "gtq", "betq", "bg2"]}
    in_maps = []
    for c in range(NCORES):
        sl = slice(c * BPC, (c + 1) * BPC)
        in_maps.append({"q_loc": q[sl], "s_loc": s[sl], **w16, **w32})
    return in_maps


def kernel(**inputs):
    nc = _get_nc()
    in_maps = make_in_maps(inputs)
    res = run_bass_kernel_spmd(nc, in_maps, core_ids=list(range(NCORES)))
    E_q = np.concatenate([res.results[c]["eq_loc"] for c in range(NCORES)],
                         axis=0).astype(np.float32)
    E_s = np.concatenate([res.results[c]["es_loc"] for c in range(NCORES)],
                         axis=0).astype(np.float32)
    return E_q, E_s
